# revision 1
# baseline (speedup 1.0000x reference)
"""Distributed 2-layer GAT on 8 Trainium2 NeuronCores (Bass/Tile).

Strategy (graph/data parallel, per sharding hint):
  - Nodes are sharded across 8 cores (6250 each, padded to 6272 = 49*128).
  - Within a core, nodes are greedily packed into 49 tiles of 128 so that
    per-tile in-edge counts are balanced (pad edges fill to K*128).
  - Layer tables ([h | al_src] per node) are computed locally per shard
    (x @ [W1 | W1@a_src | W1@a_dst] extended matmul) and AllGathered so
    every core holds the full node table in DRAM.
  - Edge pass per destination tile: indirect-DMA gather of source rows,
    attention weights ex = exp(leaky_relu(al_src[src]+al_dst[dst]))
    (unnormalized softmax - mathematically identical to the max-subtracted
    reference), weighted rows scatter-added into PSUM via a one-hot
    selection matmul; appended ex columns accumulate the softmax
    denominators in the same matmul. Per-node divide afterwards.
  - Dummy (pad) edges point their al_dst gather at a -1e9 row -> ex == 0.
"""

import heapq
import os
import sys
import types

import numpy as np

_BUILD_CACHE = {}


def _register_trace_hook():
    """Best-effort registration of the axon NTFF profiling hook."""
    try:
        if "antenv.axon_hooks" in sys.modules:
            return True
        from trn_agent_boot.trn_boot import _ntff_profile_via_ctypes

        hook = _ntff_profile_via_ctypes("/opt/axon/libaxon_pjrt.so")
        m = types.ModuleType("antenv.axon_hooks")
        m.get_axon_ntff_profile_hook = lambda: hook
        m.set_axon_ntff_profile_hook = lambda h: None
        sys.modules["antenv.axon_hooks"] = m
        return True
    except Exception:
        return False


def _host_prep(x, edge_index, W1, a_src1, a_dst1, b1, W2, a_src2, a_dst2, b2, C):
    x = np.asarray(x, np.float32)
    ei = np.asarray(edge_index)
    W1 = np.asarray(W1, np.float32)
    a_src1 = np.asarray(a_src1, np.float32)
    a_dst1 = np.asarray(a_dst1, np.float32)
    b1 = np.asarray(b1, np.float32)
    W2 = np.asarray(W2, np.float32)
    a_src2 = np.asarray(a_src2, np.float32)
    a_dst2 = np.asarray(a_dst2, np.float32)
    b2 = np.asarray(b2, np.float32)

    N, F = x.shape
    HEADS, HID = a_src1.shape
    D1 = HEADS * HID
    NCLS = W2.shape[1]
    assert N % C == 0
    NPC = N // C
    NT = -(-NPC // 128)
    PADN = NT * 128
    assert PADN > NPC, "need at least one pad slot per core for dummy rows"
    NPALL = C * PADN
    E = ei.shape[1]
    ET = E + N

    src = np.concatenate([ei[0], np.arange(N)]).astype(np.int64)
    dst = np.concatenate([ei[1], np.arange(N)]).astype(np.int64)

    # --- greedy degree-balanced node->tile assignment per core ---
    deg = np.bincount(dst, minlength=N).astype(np.int64)
    pos = np.empty(N, np.int64)
    for c in range(C):
        lo = c * NPC
        d = deg[lo:lo + NPC]
        order = np.argsort(-d, kind="stable")
        cnts = np.zeros(NT, np.int64)
        heap = [(0, t) for t in range(NT)]
        heapq.heapify(heap)
        ploc = np.empty(NPC, np.int64)
        for i in order:
            while True:
                load, t = heapq.heappop(heap)
                if cnts[t] < 128:
                    break
            ploc[i] = t * 128 + cnts[t]
            cnts[t] += 1
            if cnts[t] < 128:
                heapq.heappush(heap, (load + d[i], t))
        pos[lo:lo + NPC] = ploc

    ncidx = np.arange(N) // NPC
    node_at = np.full((C, PADN), -1, np.int64)
    node_at[ncidx, pos] = np.arange(N)
    grow = ncidx * PADN + pos  # global table row per node

    padrow = np.empty(C, np.int64)
    for c in range(C):
        w = np.where(node_at[c] < 0)[0]
        padrow[c] = c * PADN + w[0]

    # --- edge grouping by (dst core, dst tile) ---
    ec = dst // NPC
    et = pos[dst] // 128
    gkey = ec * NT + et
    # within each (core, tile) group, order edges by source table row so each
    # gather call's descriptors hit monotonically increasing DRAM addresses
    order_e = np.lexsort((pos[src] + (src // NPC) * PADN, gkey))
    ks = gkey[order_e]
    gstart = np.searchsorted(ks, np.arange(C * NT))
    gcnt = np.searchsorted(ks, np.arange(C * NT) + 1) - gstart
    K = int(-(-gcnt.max() // 128))
    jj = np.arange(ET) - gstart[ks]
    kk_e = jj // 128
    pp_e = jj % 128
    cc_e = ks // NT
    tt_e = ks % NT

    s_src = src[order_e]
    s_dst = dst[order_e]
    srcg = np.empty((C, NT, 128, K), np.int32)
    srcg[...] = padrow[:, None, None, None]
    dstl = np.full((C, NT, 128, K), 999.0, np.float32)
    srcg[cc_e, tt_e, pp_e, kk_e] = grow[s_src]
    dstl[cc_e, tt_e, pp_e, kk_e] = (pos[s_dst] % 128).astype(np.float32)

    # --- per-core transposed x shards (pad rows zero) ---
    xs = np.zeros((C, PADN, F), np.float32)
    xs[ncidx, pos] = x
    xsT = np.ascontiguousarray(xs.transpose(0, 2, 1))

    # --- extended weights ---
    Wa_s1 = np.einsum("fhc,hc->fh", W1.reshape(F, HEADS, HID), a_src1)
    Wa_d1 = np.einsum("fhc,hc->fh", W1.reshape(F, HEADS, HID), a_dst1)
    W1e = np.ascontiguousarray(
        np.concatenate([W1, Wa_s1, Wa_d1], axis=1), dtype=np.float32
    )
    Wa_s2 = W2 @ a_src2[0]
    Wa_d2 = W2 @ a_dst2[0]
    W2e = np.ascontiguousarray(
        np.concatenate([W2, Wa_s2[:, None], Wa_d2[:, None]], axis=1),
        dtype=np.float32,
    )

    # replicated-transposed dstl for the PE al_dst broadcast: [t, k, j, e] = dstl[t, e, k]
    dstlr = np.ascontiguousarray(
        np.broadcast_to(dstl.transpose(0, 1, 3, 2)[:, :, :, None, :],
                        (C, NT, K, 128, 128)), np.float32)
    iotac = np.arange(128, dtype=np.float32)[:, None].copy()
    b1r = np.ascontiguousarray(np.broadcast_to(b1[None, :], (128, D1)), np.float32)
    b2r = np.ascontiguousarray(np.broadcast_to(b2[None, :], (128, NCLS)), np.float32)
    iota = np.ascontiguousarray(
        np.broadcast_to(np.arange(128, dtype=np.float32)[None, :], (128, 128))
    )
    ident = np.eye(128, dtype=np.float32)

    cfg = dict(F=F, HEADS=HEADS, HID=HID, D1=D1, NCLS=NCLS, NT=NT, PADN=PADN,
               NPALL=NPALL, K=K, C=C)
    in_maps = []
    for c in range(C):
        in_maps.append({
            "xsT": xsT[c],
            "w1e": W1e,
            "w2e": W2e,
            "b1r": b1r,
            "b2r": b2r,
            "iota": iota,
            "ident": ident,
            "srcg": srcg[c],
            "dstl": dstl[c],
            "dstlr": dstlr[c],
            "iotac": iotac,
        })
    return cfg, in_maps, node_at, (N, NCLS)


def _build_program(F, HEADS, HID, D1, NCLS, NT, PADN, NPALL, K, C):
    import concourse.bacc as bacc
    import concourse.bass as bass
    import concourse.mybir as mybir
    import concourse.tile as tile

    f32 = mybir.dt.float32
    bf16 = mybir.dt.bfloat16
    i32 = mybir.dt.int32
    AF = mybir.ActivationFunctionType
    ALU = mybir.AluOpType
    AX = mybir.AxisListType

    TW1 = D1 + 2 * HEADS          # local layer-1 matmul width
    G1W = D1 + HEADS              # layer-1 gather row width [h | al_src]
    TW2 = NCLS + 2                # layer-2 table width [z2 | al_src2 | al_dst2]
    G2W = NCLS + 1                # layer-2 gather width [z2 | al_src2]
    FK = F // 128
    DK = D1 // 128

    nc = bacc.Bacc("TRN2", target_bir_lowering=False, debug=False, num_devices=C)

    xsT = nc.dram_tensor("xsT", [F, PADN], f32, kind="ExternalInput")
    w1e = nc.dram_tensor("w1e", [F, TW1], f32, kind="ExternalInput")
    w2e = nc.dram_tensor("w2e", [D1, TW2], f32, kind="ExternalInput")
    b1r = nc.dram_tensor("b1r", [128, D1], f32, kind="ExternalInput")
    b2r = nc.dram_tensor("b2r", [128, NCLS], f32, kind="ExternalInput")
    iot = nc.dram_tensor("iota", [128, 128], f32, kind="ExternalInput")
    idn = nc.dram_tensor("ident", [128, 128], f32, kind="ExternalInput")
    srcg = nc.dram_tensor("srcg", [NT, 128, K], i32, kind="ExternalInput")
    dstlr = nc.dram_tensor("dstlr", [NT, K, 128, 128], f32, kind="ExternalInput")
    iotac = nc.dram_tensor("iotac", [128, 1], f32, kind="ExternalInput")
    dstl = nc.dram_tensor("dstl", [NT, 128, K], f32, kind="ExternalInput")
    outp = nc.dram_tensor("outp", [PADN, NCLS], f32, kind="ExternalOutput")

    loc1 = nc.dram_tensor("loc1", [PADN, G1W], f32)
    tab1 = nc.dram_tensor("tab1", [NPALL, G1W], f32, addr_space="Shared")
    ald1 = nc.dram_tensor("ald1", [PADN + 1, HEADS], f32)
    loc2 = nc.dram_tensor("loc2", [PADN, TW2], f32)
    tab2 = nc.dram_tensor("tab2", [NPALL, TW2], f32, addr_space="Shared")
    ald2 = nc.dram_tensor("ald2", [PADN + 1, 1], f32)

    rg = [list(range(C))]

    with tile.TileContext(nc) as tc:
        with (
            tc.tile_pool(name="const", bufs=1) as const,
            tc.tile_pool(name="h2", bufs=1) as h2p,
            tc.tile_pool(name="shp", bufs=1) as shp,
            tc.tile_pool(name="wk", bufs=2) as wk,
            tc.tile_pool(name="idx", bufs=3) as idxp,
            tc.tile_pool(name="ps", bufs=2, space="PSUM") as psp,
        ):
            # ---- constants ----
            w1t = []
            for kk in range(FK):
                t_ = const.tile([128, TW1], f32, tag=f"w1_{kk}")
                nc.sync.dma_start(out=t_[:], in_=w1e[kk * 128:(kk + 1) * 128, :])
                w1t.append(t_)
            w2t = []
            for kk in range(DK):
                t_ = const.tile([128, TW2], f32, tag=f"w2_{kk}")
                nc.sync.dma_start(out=t_[:], in_=w2e[kk * 128:(kk + 1) * 128, :])
                w2t.append(t_)
            b1s = const.tile([128, D1], f32, tag="b1")
            nc.sync.dma_start(out=b1s[:], in_=b1r[:, :])
            b2s = const.tile([128, NCLS], f32, tag="b2")
            nc.sync.dma_start(out=b2s[:], in_=b2r[:, :])
            ios = const.tile([128, 128], f32, tag="iota")
            nc.sync.dma_start(out=ios[:], in_=iot[:, :])
            ids = const.tile([128, 128], f32, tag="ident")
            nc.sync.dma_start(out=ids[:], in_=idn[:, :])
            neg1 = const.tile([1, HEADS], f32, tag="neg1")
            nc.vector.memset(neg1[:], -1e9)
            nc.sync.dma_start(out=ald1[PADN:PADN + 1, :], in_=neg1[:])
            neg2 = const.tile([1, 1], f32, tag="neg2")
            nc.vector.memset(neg2[:], -1e9)
            nc.sync.dma_start(out=ald2[PADN:PADN + 1, :], in_=neg2[:])
            ioc = const.tile([128, 1], f32, tag="ioc")
            nc.sync.dma_start(out=ioc[:], in_=iotac[:, :])
            ssum = const.tile([128, NT], f32, tag="ssum")
            lgs = const.tile([128, NT], f32, tag="lgs")

            # ---- phase A: local h = x @ [W1 | Wa_src | Wa_dst] ----
            with nc.named_scope("l1_local_mm"):
                for t in range(NT):
                    ps_a = psp.tile([128, TW1], f32, tag="mm")
                    for kk in range(FK):
                        xt = wk.tile([128, 128], f32, tag=f"xt{kk}")
                        nc.sync.dma_start(
                            out=xt[:],
                            in_=xsT[kk * 128:(kk + 1) * 128, t * 128:(t + 1) * 128],
                        )
                        nc.tensor.matmul(ps_a[:], lhsT=xt[:], rhs=w1t[kk][:],
                                         start=(kk == 0), stop=(kk == FK - 1))
                    ha = wk.tile([128, TW1], f32, tag="ha")
                    nc.scalar.copy(ha[:], ps_a[:])
                    nc.sync.dma_start(out=loc1[t * 128:(t + 1) * 128, :],
                                      in_=ha[:, 0:G1W])
                    nc.sync.dma_start(out=ald1[t * 128:(t + 1) * 128, :],
                                      in_=ha[:, D1 + HEADS:D1 + 2 * HEADS])

            # ---- phase B: allgather layer-1 table ----
            with nc.named_scope("l1_allgather"):
                nc.gpsimd.collective_compute(
                    "AllGather", mybir.AluOpType.bypass, replica_groups=rg,
                    ins=[loc1[:]], outs=[tab1[:]],
                )
            tc.strict_bb_all_engine_barrier()

            # ---- phase C: layer-1 edge pass ----
            o1_tiles = []
            with nc.named_scope("l1_edges"):
                for t in range(NT):
                    sg = idxp.tile([128, K], i32, tag="sg")
                    nc.sync.dma_start(out=sg[:], in_=srcg[t])
                    dl = idxp.tile([128, K], f32, tag="dl")
                    nc.sync.dma_start(out=dl[:], in_=dstl[t])
                    alt = idxp.tile([128, HEADS], f32, tag="alt")
                    nc.sync.dma_start(out=alt[:], in_=ald1[t * 128:(t + 1) * 128, :])

                    g1 = wk.tile([128, K * G1W], f32, tag="g1")
                    g1v = g1[:].rearrange("p (k c) -> p k c", c=G1W)
                    for k in range(K):
                        nc.gpsimd.indirect_dma_start(
                            out=g1v[:, k, :], out_offset=None, in_=tab1[:, :],
                            in_offset=bass.IndirectOffsetOnAxis(
                                ap=sg[:, k:k + 1], axis=0),
                        )
                    # al_dst per edge via transposed-onehot matmul
                    ps_e = psp.tile([128, K * HEADS], f32, tag="ed")
                    for k in range(K):
                        dr = wk.tile([128, 128], f32, tag="dr")
                        nc.sync.dma_start(out=dr[:], in_=dstlr[t, k])
                        ohT = wk.tile([128, 128], f32, tag="ohT")
                        nc.vector.tensor_scalar(ohT[:], dr[:], ioc[:, 0:1], None,
                                                op0=ALU.is_equal)
                        nc.tensor.matmul(ps_e[:, k * HEADS:(k + 1) * HEADS],
                                         lhsT=ohT[:], rhs=alt[:],
                                         start=(k == 0), stop=(k == K - 1))

                    e1 = idxp.tile([128, K * HEADS], f32, tag="e1")
                    e1v = e1[:].rearrange("p (k h) -> p k h", h=HEADS)
                    nc.vector.tensor_add(
                        e1v, g1v[:, :, D1:D1 + HEADS],
                        ps_e[:].rearrange("p (k h) -> p k h", h=HEADS))
                    lr = idxp.tile([128, K * HEADS], f32, tag="lr")
                    nc.vector.tensor_scalar_mul(lr[:], e1[:], 0.2)
                    nc.vector.tensor_max(lr[:], lr[:], e1[:])
                    exw = idxp.tile([128, K * HEADS], f32, tag="exw")
                    nc.scalar.activation(exw[:], lr[:], AF.Exp)
                    exv = exw[:].rearrange("p (k h) -> p k h", h=HEADS)

                    # weight gathered rows in place; ex into the al_src cols
                    g1f = g1v[:, :, 0:D1].rearrange("p k (h c) -> p k h c", c=HID)
                    exb = exv.unsqueeze(3).to_broadcast([128, K, HEADS, HID])
                    nc.vector.tensor_mul(g1f, g1f, exb)
                    nc.vector.tensor_copy(g1v[:, :, D1:D1 + HEADS], exv)

                    oh = wk.tile([128, K * 128], f32, tag="oh")
                    ohv = oh[:].rearrange("p (k j) -> p k j", j=128)
                    dlb = dl[:].unsqueeze(2).to_broadcast([128, K, 128])
                    iob = ios[:].unsqueeze(1).to_broadcast([128, K, 128])
                    nc.vector.tensor_tensor(ohv, dlb, iob, op=ALU.is_equal)

                    ps_c = psp.tile([128, TW1], f32, tag="mm")
                    for k in range(K):
                        nc.tensor.matmul(
                            ps_c[:, 0:G1W],
                            lhsT=oh[:, k * 128:(k + 1) * 128],
                            rhs=g1[:, k * G1W:(k + 1) * G1W],
                            start=(k == 0), stop=(k == K - 1),
                        )

                    den = idxp.tile([128, HEADS], f32, tag="den")
                    nc.vector.tensor_scalar_add(den[:], ps_c[:, D1:D1 + HEADS], 1e-16)
                    rec = idxp.tile([128, HEADS], f32, tag="rec")
                    nc.vector.reciprocal(rec[:], den[:])

                    o1 = h2p.tile([128, D1], f32, tag=f"h2_{t}")
                    o1v = o1[:].rearrange("p (h c) -> p h c", c=HID)
                    recb = rec[:].unsqueeze(2).to_broadcast([128, HEADS, HID])
                    psf = ps_c[:, 0:D1].rearrange("p (h c) -> p h c", c=HID)
                    nc.vector.tensor_mul(o1v, psf, recb)
                    nc.vector.tensor_add(o1[:], o1[:], b1s[:])
                    # elu(x) = max(x,0) + exp(min(x,0)) - 1
                    tn = wk.tile([128, D1], f32, tag="tn")
                    nc.vector.tensor_scalar_min(tn[:], o1[:], 0.0)
                    nc.scalar.activation(tn[:], tn[:], AF.Exp)
                    nc.vector.tensor_scalar_max(o1[:], o1[:], 0.0)
                    nc.vector.tensor_add(o1[:], o1[:], tn[:])
                    nc.vector.tensor_scalar_add(o1[:], o1[:], -1.0)
                    o1_tiles.append(o1)

            # ---- phase D: layer-2 local z2 = h2 @ [W2 | Wa2_src | Wa2_dst] ----
            with nc.named_scope("l2_local_mm"):
                for t in range(NT):
                    tts = []
                    for kk in range(DK):
                        ps_t = psp.tile([128, 128], f32, tag="tr")
                        nc.tensor.transpose(
                            ps_t[:], o1_tiles[t][:, kk * 128:(kk + 1) * 128], ids[:]
                        )
                        tt = wk.tile([128, 128], f32, tag=f"tt{kk}")
                        nc.scalar.copy(tt[:], ps_t[:])
                        tts.append(tt)
                    ps_d = psp.tile([128, TW2], f32, tag="mm")
                    for kk in range(DK):
                        nc.tensor.matmul(ps_d[:], lhsT=tts[kk][:], rhs=w2t[kk][:],
                                         start=(kk == 0), stop=(kk == DK - 1))
                    hd = wk.tile([128, TW2], f32, tag="hd")
                    nc.scalar.copy(hd[:], ps_d[:])
                    nc.sync.dma_start(out=loc2[t * 128:(t + 1) * 128, :],
                                      in_=hd[:, 0:TW2])
                    nc.sync.dma_start(out=ald2[t * 128:(t + 1) * 128, :],
                                      in_=hd[:, TW2 - 1:TW2])

            # ---- phase E: allgather layer-2 table ----
            with nc.named_scope("l2_allgather"):
                nc.gpsimd.collective_compute(
                    "AllGather", mybir.AluOpType.bypass, replica_groups=rg,
                    ins=[loc2[:]], outs=[tab2[:]],
                )
            tc.strict_bb_all_engine_barrier()

            # ---- phase F: layer-2 edge pass ----
            sh_tiles = []
            with nc.named_scope("l2_edges"):
                for t in range(NT):
                    sg = idxp.tile([128, K], i32, tag="sg")
                    nc.sync.dma_start(out=sg[:], in_=srcg[t])
                    dl = idxp.tile([128, K], f32, tag="dl")
                    nc.sync.dma_start(out=dl[:], in_=dstl[t])
                    alt2 = idxp.tile([128, 1], f32, tag="alt2")
                    nc.sync.dma_start(out=alt2[:], in_=ald2[t * 128:(t + 1) * 128, :])

                    g2 = wk.tile([128, K * G2W], f32, tag="g2")
                    g2v = g2[:].rearrange("p (k c) -> p k c", c=G2W)
                    for k in range(K):
                        nc.gpsimd.indirect_dma_start(
                            out=g2v[:, k, :], out_offset=None, in_=tab2[:, :],
                            in_offset=bass.IndirectOffsetOnAxis(
                                ap=sg[:, k:k + 1], axis=0),
                        )
                    ps_e2 = psp.tile([128, K * HEADS], f32, tag="ed")
                    for k in range(K):
                        dr = wk.tile([128, 128], f32, tag="dr")
                        nc.sync.dma_start(out=dr[:], in_=dstlr[t, k])
                        ohT = wk.tile([128, 128], f32, tag="ohT")
                        nc.vector.tensor_scalar(ohT[:], dr[:], ioc[:, 0:1], None,
                                                op0=ALU.is_equal)
                        nc.tensor.matmul(ps_e2[:, k:k + 1],
                                         lhsT=ohT[:], rhs=alt2[:],
                                         start=(k == 0), stop=(k == K - 1))

                    e2 = idxp.tile([128, K], f32, tag="e2")
                    nc.vector.tensor_add(e2[:], g2v[:, :, NCLS], ps_e2[:, 0:K])
                    lr2 = idxp.tile([128, K], f32, tag="lr2")
                    nc.vector.tensor_scalar_mul(lr2[:], e2[:], 0.2)
                    nc.vector.tensor_max(lr2[:], lr2[:], e2[:])
                    ex2 = idxp.tile([128, K], f32, tag="ex2")
                    nc.scalar.activation(ex2[:], lr2[:], AF.Exp)

                    g2f = g2v[:, :, 0:NCLS]
                    ex2b = ex2[:].unsqueeze(2).to_broadcast([128, K, NCLS])
                    nc.vector.tensor_mul(g2f, g2f, ex2b)
                    nc.vector.tensor_copy(g2v[:, :, NCLS], ex2[:])

                    oh = wk.tile([128, K * 128], f32, tag="oh")
                    ohv = oh[:].rearrange("p (k j) -> p k j", j=128)
                    dlb = dl[:].unsqueeze(2).to_broadcast([128, K, 128])
                    iob = ios[:].unsqueeze(1).to_broadcast([128, K, 128])
                    nc.vector.tensor_tensor(ohv, dlb, iob, op=ALU.is_equal)

                    ps_f = psp.tile([128, TW1], f32, tag="mm")
                    for k in range(K):
                        nc.tensor.matmul(
                            ps_f[:, 0:G2W],
                            lhsT=oh[:, k * 128:(k + 1) * 128],
                            rhs=g2[:, k * G2W:(k + 1) * G2W],
                            start=(k == 0), stop=(k == K - 1),
                        )

                    den2 = idxp.tile([128, 1], f32, tag="den2")
                    nc.vector.tensor_scalar_add(den2[:], ps_f[:, NCLS:NCLS + 1], 1e-16)
                    rec2 = idxp.tile([128, 1], f32, tag="rec2")
                    nc.vector.reciprocal(rec2[:], den2[:])

                    o2 = wk.tile([128, NCLS], f32, tag="o2")
                    nc.vector.tensor_scalar(o2[:], ps_f[:, 0:NCLS], rec2[:], None,
                                            op0=ALU.mult)
                    nc.vector.tensor_add(o2[:], o2[:], b2s[:])

                    rmax = idxp.tile([128, 1], f32, tag="rmax")
                    nc.vector.reduce_max(rmax[:], o2[:], axis=AX.X)
                    sh = shp.tile([128, NCLS], f32, tag=f"sh_{t}")
                    nc.vector.tensor_scalar(sh[:], o2[:], rmax[:], None,
                                            op0=ALU.subtract)
                    exs = wk.tile([128, NCLS], f32, tag="exs")
                    nc.scalar.activation(exs[:], sh[:], AF.Exp)
                    nc.vector.reduce_sum(ssum[:, t:t + 1], exs[:], axis=AX.X)
                    sh_tiles.append(sh)

            # ---- phase G: log-softmax finalize ----
            with nc.named_scope("logsoftmax"):
                nc.scalar.activation(lgs[:], ssum[:], AF.Ln)
                for t in range(NT):
                    outf = wk.tile([128, NCLS], f32, tag="outf")
                    nc.vector.tensor_scalar(outf[:], sh_tiles[t][:], lgs[:, t:t + 1],
                                            None, op0=ALU.subtract)
                    nc.sync.dma_start(out=outp[t * 128:(t + 1) * 128, :], in_=outf[:])

    nc.compile()
    return nc


def _get_program(cfg):
    key = tuple(sorted(cfg.items()))
    if key not in _BUILD_CACHE:
        _BUILD_CACHE[key] = _build_program(**cfg)
    return _BUILD_CACHE[key]


def kernel(**inputs):
    C = 8
    cfg, in_maps, node_at, (N, NCLS) = _host_prep(
        inputs["x"], inputs["edge_index"], inputs["W1"], inputs["a_src1"],
        inputs["a_dst1"], inputs["b1"], inputs["W2"], inputs["a_src2"],
        inputs["a_dst2"], inputs["b2"], C,
    )
    nc = _get_program(cfg)

    from concourse.bass_utils import run_bass_kernel_spmd

    trace = bool(int(os.environ.get("GAT_PROFILE", "0")))
    if trace:
        trace = _register_trace_hook()
    res = run_bass_kernel_spmd(nc, in_maps, list(range(C)), trace=trace)
    if trace and res.exec_time_ns is not None:
        print(f"HW exec time: {res.exec_time_ns} ns", flush=True)

    out = np.empty((N, NCLS), np.float32)
    for c in range(C):
        r = res.results[c]["outp"]
        m = node_at[c] >= 0
        out[node_at[c][m]] = r[m]
    return out



# revision 7
# speedup vs baseline: 1.9277x; 1.9277x over previous
"""Distributed 2-layer GAT on 8 Trainium2 NeuronCores (Bass/Tile).

Strategy (graph/data parallel, per sharding hint):
  - Nodes sharded across 8 cores (6250 each, padded to 6272 = 49*128); nodes
    greedily packed into 49 tiles of 128 balancing per-tile in-edge counts.
  - Layer tables (bf16) computed locally ([h | al_src | al_dst] extended
    matmul) and AllGathered so every core holds the full node table in DRAM.
    Table rows padded to 256B multiples for the ucode gather (768B layer 1,
    256B layer 2).
  - Edge pass per destination tile: one ucode dma_gather per (tile, src-range
    half) pulls all remote source rows (int16 indices force a <32768 / >=32768
    range split; edges sorted by source row so gather columns are range-pure;
    trailing -1 indices + per-core exact counts loaded into a gpsimd register
    skip pad descriptors). Self-loops bypass the gather: the tile's own rows
    are fetched with one contiguous DMA and accumulated via an identity
    matmul. Attention: al_dst broadcast to edge slots via transposed one-hot
    matmuls (one-hot built on-chip, PE-transposed). Unnormalized softmax
    ex = exp(leaky_relu(al_src+al_dst)) scales the gathered rows; a one-hot
    selection matmul scatter-adds rows and denominators into PSUM.
  - Pad edge slots keep dst-slot 999 -> zero one-hot column -> no
    contribution (gather buffers are zero-primed so skipped slots hold
    finite stale data).
  - Layer-2 local matmul fused into the layer-1 edge loop (elu(x)+1 fed
    through W2 with a -colsum(W2e) correction row); final log-softmax fused
    into the layer-2 edge loop.
"""

import heapq
import os
import sys
import types

import ml_dtypes
import numpy as np

_BUILD_CACHE = {}
LOWR = 32768


def _register_trace_hook():
    """Best-effort registration of the axon NTFF profiling hook."""
    try:
        if "antenv.axon_hooks" in sys.modules:
            return True
        from trn_agent_boot.trn_boot import _ntff_profile_via_ctypes

        hook = _ntff_profile_via_ctypes("/opt/axon/libaxon_pjrt.so")
        m = types.ModuleType("antenv.axon_hooks")
        m.get_axon_ntff_profile_hook = lambda: hook
        m.set_axon_ntff_profile_hook = lambda h: None
        sys.modules["antenv.axon_hooks"] = m
        return True
    except Exception:
        return False


def _host_prep(x, edge_index, W1, a_src1, a_dst1, b1, W2, a_src2, a_dst2, b2, C):
    bf16 = ml_dtypes.bfloat16
    x = np.asarray(x, np.float32)
    ei = np.asarray(edge_index)
    W1 = np.asarray(W1, np.float32)
    a_src1 = np.asarray(a_src1, np.float32)
    a_dst1 = np.asarray(a_dst1, np.float32)
    b1 = np.asarray(b1, np.float32)
    W2 = np.asarray(W2, np.float32)
    a_src2 = np.asarray(a_src2, np.float32)
    a_dst2 = np.asarray(a_dst2, np.float32)
    b2 = np.asarray(b2, np.float32)

    N, F = x.shape
    HEADS, HID = a_src1.shape
    D1 = HEADS * HID
    NCLS = W2.shape[1]
    assert N % C == 0
    NPC = N // C
    NT = -(-NPC // 128)
    PADN = NT * 128
    NPALL = C * PADN
    E = ei.shape[1]

    # self-loops are handled separately on-chip; only real edges here
    src = ei[0].astype(np.int64)
    dst = ei[1].astype(np.int64)

    # --- greedy degree-balanced node->tile assignment per core ---
    deg = np.bincount(dst, minlength=N).astype(np.int64)
    pos = np.empty(N, np.int64)
    for c in range(C):
        lo = c * NPC
        d = deg[lo:lo + NPC]
        order = np.argsort(-d, kind="stable")
        cnts_ = np.zeros(NT, np.int64)
        heap = [(0, t) for t in range(NT)]
        heapq.heapify(heap)
        ploc = np.empty(NPC, np.int64)
        for i in order:
            while True:
                load, t = heapq.heappop(heap)
                if cnts_[t] < 128:
                    break
            ploc[i] = t * 128 + cnts_[t]
            cnts_[t] += 1
            if cnts_[t] < 128:
                heapq.heappush(heap, (load + d[i], t))
        pos[lo:lo + NPC] = ploc

    ncidx = np.arange(N) // NPC
    node_at = np.full((C, PADN), -1, np.int64)
    node_at[ncidx, pos] = np.arange(N)
    grow = ncidx * PADN + pos  # global table row per node

    # --- edge grouping by (dst core, dst tile, src-range half), src-sorted ---
    ec = dst // NPC
    et = pos[dst] // 128
    srow = grow[src]
    half = (srow >= LOWR).astype(np.int64)
    gkey = (ec * NT + et) * 2 + half
    order_e = np.lexsort((srow, gkey))
    ks = gkey[order_e]
    cnt = np.bincount(ks, minlength=C * NT * 2).reshape(C, NT, 2)
    KL = int(-(-cnt[:, :, 0].max() // 128))
    KH = int(-(-cnt[:, :, 1].max() // 128))
    KT = KL + KH

    gstart = np.searchsorted(ks, np.arange(C * NT * 2))
    jj = np.arange(E) - gstart[ks]
    half_s = ks % 2
    col = jj // 128 + np.where(half_s, KL, 0)
    pp = jj % 128
    cc = ks // (2 * NT)
    tt = (ks // 2) % NT

    s_dst = dst[order_e]
    idxf = np.zeros((C, NT, KT, 128), np.int16)
    idxf[cc, tt, col, pp] = (srow[order_e] - np.where(half_s, LOWR, 0)).astype(np.int16)
    dlarr = np.full((C, NT, KT, 128), 999.0, np.float32)
    dlarr[cc, tt, col, pp] = (pos[s_dst] % 128).astype(np.float32)

    # wrapped int16 index layout: call index i -> [i % 16, i // 16]
    def wrap(a, K):  # a: [C, NT, K*128] -> [C, NT, 16, K*8]
        Cn, NTn, NI = a.shape
        w = np.zeros((Cn, NTn, 16, NI // 16), np.int16)
        i = np.arange(NI)
        w[:, :, i % 16, i // 16] = a
        return w

    wl = wrap(idxf[:, :, 0:KL, :].reshape(C, NT, KL * 128), KL)
    wh = wrap(idxf[:, :, KL:KT, :].reshape(C, NT, KH * 128), KH)
    wfull = np.concatenate([wl, wh], axis=3)  # [C, NT, 16, KT*8]
    wfull = wfull.transpose(0, 2, 1, 3).reshape(C, 16, NT * KT * 8)
    ixs = np.tile(wfull, (1, 8, 1))  # [C, 128, NT*KT*8]

    dls = np.ascontiguousarray(
        dlarr.transpose(0, 3, 1, 2).reshape(C, 128, NT * KT)
    ).astype(bf16)
    cnts = cnt.reshape(C, 1, NT * 2).astype(np.int32)

    # --- per-core per-tile transposed x blocks (pad rows zero) ---
    xs = np.zeros((C, PADN, F), np.float32)
    xs[ncidx, pos] = x
    FK = F // 128
    xsTt = np.ascontiguousarray(
        xs.reshape(C, NT, 128, FK, 128).transpose(0, 1, 3, 4, 2)
    ).astype(bf16)  # [C, NT, FK, 128(feat), 128(node)]

    # --- extended weights ---
    Wa_s1 = np.einsum("fhc,hc->fh", W1.reshape(F, HEADS, HID), a_src1)
    Wa_d1 = np.einsum("fhc,hc->fh", W1.reshape(F, HEADS, HID), a_dst1)
    W1e = np.concatenate([W1, Wa_s1, Wa_d1], axis=1).astype(bf16)
    Wa_s2 = W2 @ a_src2[0]
    Wa_d2 = W2 @ a_dst2[0]
    W2e_f = np.concatenate([W2, Wa_s2[:, None], Wa_d2[:, None]], axis=1)
    W2e = W2e_f.astype(bf16)
    w2corr = (-W2e_f.sum(axis=0, keepdims=True)).astype(bf16)

    b1r = np.ascontiguousarray(np.broadcast_to(b1[None, :], (128, D1)), np.float32)
    b2r = np.ascontiguousarray(np.broadcast_to(b2[None, :], (128, NCLS)), np.float32)
    iotab = np.ascontiguousarray(
        np.broadcast_to(np.arange(128, dtype=np.float32)[None, :], (128, 128))
    ).astype(bf16)
    identb = np.eye(128, dtype=np.float32).astype(bf16)
    onesb = np.ones((1, 128), np.float32).astype(bf16)

    cfg = dict(F=F, HEADS=HEADS, HID=HID, D1=D1, NCLS=NCLS, NT=NT, PADN=PADN,
               NPALL=NPALL, KL=KL, KH=KH, C=C)
    in_maps = []
    for c in range(C):
        in_maps.append({
            "xsTt": xsTt[c],
            "w1e": W1e,
            "w2e": W2e,
            "w2c": w2corr,
            "b1r": b1r,
            "b2r": b2r,
            "iotab": iotab,
            "identb": identb,
            "onesb": onesb,
            "ixs": ixs[c],
            "dls": dls[c],
            "cnts": cnts[c],
        })
    return cfg, in_maps, node_at, (N, NCLS)


def _build_program(F, HEADS, HID, D1, NCLS, NT, PADN, NPALL, KL, KH, C):
    import concourse.bacc as bacc
    import concourse.bass as bass
    import concourse.mybir as mybir
    import concourse.tile as tile

    f32 = mybir.dt.float32
    bf16 = mybir.dt.bfloat16
    i16 = mybir.dt.int16
    i32 = mybir.dt.int32
    AF = mybir.ActivationFunctionType
    ALU = mybir.AluOpType
    AX = mybir.AxisListType

    KT = KL + KH
    TW1 = D1 + 2 * HEADS          # 264: [h | al_src | al_dst]
    G1W = D1 + HEADS              # 260: gathered layer-1 row payload
    T1W = 384                     # layer-1 table row (768B, 256B multiple)
    TW2 = NCLS + 2                # 66: [z2 | al_src2 | al_dst2]
    G2W = NCLS + 1                # 65: gathered layer-2 row payload
    T2W = 128                     # layer-2 table row (256B)
    FK = F // 128
    DK = D1 // 128

    nc = bacc.Bacc("TRN2", target_bir_lowering=False, debug=False, num_devices=C)

    xsTt = nc.dram_tensor("xsTt", [NT, FK, 128, 128], bf16, kind="ExternalInput")
    w1e = nc.dram_tensor("w1e", [F, TW1], bf16, kind="ExternalInput")
    w2e = nc.dram_tensor("w2e", [D1, TW2], bf16, kind="ExternalInput")
    w2c = nc.dram_tensor("w2c", [1, TW2], bf16, kind="ExternalInput")
    b1r = nc.dram_tensor("b1r", [128, D1], f32, kind="ExternalInput")
    b2r = nc.dram_tensor("b2r", [128, NCLS], f32, kind="ExternalInput")
    iotab = nc.dram_tensor("iotab", [128, 128], bf16, kind="ExternalInput")
    identb = nc.dram_tensor("identb", [128, 128], bf16, kind="ExternalInput")
    onesb = nc.dram_tensor("onesb", [1, 128], bf16, kind="ExternalInput")
    ixsd = nc.dram_tensor("ixs", [128, NT * KT * 8], i16, kind="ExternalInput")
    dlsd = nc.dram_tensor("dls", [128, NT * KT], bf16, kind="ExternalInput")
    cntd = nc.dram_tensor("cnts", [1, NT * 2], i32, kind="ExternalInput")
    outp = nc.dram_tensor("outp", [PADN, NCLS], f32, kind="ExternalOutput")

    loc1 = nc.dram_tensor("loc1", [PADN, T1W], bf16)
    tab1 = nc.dram_tensor("tab1", [NPALL, T1W], bf16)
    loc2 = nc.dram_tensor("loc2", [PADN, T2W], bf16)
    tab2 = nc.dram_tensor("tab2", [NPALL, T2W], bf16)

    rg = [list(range(C))]

    with tile.TileContext(nc) as tc:
        with (
            tc.tile_pool(name="const", bufs=1) as const,
            tc.tile_pool(name="altp", bufs=1) as altp,
            tc.tile_pool(name="wk", bufs=2) as wk,
            tc.tile_pool(name="gp", bufs=3) as gp,
        ):
            # ---- constants ----
            w1t = []
            for kk in range(FK):
                t_ = const.tile([128, TW1], bf16, tag=f"w1_{kk}")
                nc.sync.dma_start(out=t_[:], in_=w1e[kk * 128:(kk + 1) * 128, :])
                w1t.append(t_)
            w2t = []
            for kk in range(DK):
                t_ = const.tile([128, TW2], bf16, tag=f"w2_{kk}")
                nc.sync.dma_start(out=t_[:], in_=w2e[kk * 128:(kk + 1) * 128, :])
                w2t.append(t_)
            w2cs = const.tile([1, TW2], bf16, tag="w2c")
            nc.sync.dma_start(out=w2cs[:], in_=w2c[:, :])
            b1s = const.tile([128, D1], f32, tag="b1")
            nc.sync.dma_start(out=b1s[:], in_=b1r[:, :])
            b2s = const.tile([128, NCLS], f32, tag="b2")
            nc.sync.dma_start(out=b2s[:], in_=b2r[:, :])
            iob = const.tile([128, 128], bf16, tag="iota")
            nc.sync.dma_start(out=iob[:], in_=iotab[:, :])
            idb = const.tile([128, 128], bf16, tag="ident")
            nc.sync.dma_start(out=idb[:], in_=identb[:, :])
            one = const.tile([1, 128], bf16, tag="ones")
            nc.sync.dma_start(out=one[:], in_=onesb[:, :])
            ixs = const.tile([128, NT * KT * 8], i16, tag="ixs")
            nc.sync.dma_start(out=ixs[:], in_=ixsd[:, :])
            dls = const.tile([128, NT * KT], bf16, tag="dls")
            nc.sync.dma_start(out=dls[:], in_=dlsd[:, :])
            dlv = dls[:].rearrange("p (t k) -> p t k", k=KT)
            cn = const.tile([1, NT * 2], i32, tag="cnts")
            nc.sync.dma_start(out=cn[:], in_=cntd[:, :])
            alts = []
            alt2s = []

            # ---- phase A: local [h | al_src | al_dst] = x @ W1e ----
            with nc.named_scope("l1_local_mm"):
                with tc.tile_pool(name="psA", bufs=2, space="PSUM") as psA:
                    for t in range(NT):
                        ps_a = psA.tile([128, TW1], f32, tag="mm")
                        for kk in range(FK):
                            xt = wk.tile([128, 128], bf16, tag=f"xt{kk}")
                            nc.sync.dma_start(out=xt[:], in_=xsTt[t, kk])
                            nc.tensor.matmul(ps_a[:], lhsT=xt[:], rhs=w1t[kk][:],
                                             start=(kk == 0), stop=(kk == FK - 1))
                        ha = wk.tile([128, T1W], bf16, tag="ha")
                        nc.scalar.copy(ha[:, 0:G1W], ps_a[:, 0:G1W])
                        nc.vector.memset(ha[:, G1W:T1W], 0.0)
                        alt = altp.tile([128, HEADS], bf16, tag=f"alt{t}")
                        nc.scalar.copy(alt[:], ps_a[:, D1 + HEADS:D1 + 2 * HEADS])
                        alts.append(alt)
                        nc.sync.dma_start(out=loc1[t * 128:(t + 1) * 128, :],
                                          in_=ha[:])

            # ---- phase B: allgather layer-1 table ----
            with nc.named_scope("l1_allgather"):
                nc.gpsimd.collective_compute(
                    "AllGather", mybir.AluOpType.bypass, replica_groups=rg,
                    ins=[loc1[:]], outs=[tab1[:]],
                )
            tc.strict_bb_all_engine_barrier()

            # ---- phase C: layer-1 edge pass + fused layer-2 local matmul ----
            with nc.named_scope("l1_edges"):
                with (
                    tc.tile_pool(name="psT", bufs=2, space="PSUM") as psT,
                    tc.tile_pool(name="psE", bufs=2, space="PSUM") as psE,
                    tc.tile_pool(name="psC", bufs=2, space="PSUM") as psC,
                    tc.tile_pool(name="psD", bufs=2, space="PSUM") as psD,
                ):
                    for t in range(NT):
                        g1 = gp.tile([128, KT * T1W], bf16, tag="g1")
                        g1v = g1[:].rearrange("p (k c) -> p k c", c=T1W)
                        ib = t * KT * 8
                        nc.gpsimd.dma_gather(
                            g1v[:, 0:KL, :], tab1[0:LOWR, :],
                            ixs[:, ib:ib + KL * 8], KL * 128, KL * 128, T1W,
                            single_packet=False)
                        nc.gpsimd.dma_gather(
                            g1v[:, KL:KT, :], tab1[LOWR:NPALL, :],
                            ixs[:, ib + KL * 8:ib + KT * 8], KH * 128, KH * 128,
                            T1W, single_packet=False)

                        # one-hot (edge slot p -> dst row j) and its transpose
                        oh = wk.tile([128, KT * 128], bf16, tag="oh")
                        ohv = oh[:].rearrange("p (k j) -> p k j", j=128)
                        dlb = dlv[:, t, :].unsqueeze(2).to_broadcast([128, KT, 128])
                        iobb = iob[:].unsqueeze(1).to_broadcast([128, KT, 128])
                        nc.vector.tensor_tensor(ohv, dlb, iobb, op=ALU.is_equal)
                        ohT = wk.tile([128, KT * 128], bf16, tag="ohT")
                        for k in range(KT):
                            pt = psT.tile([128, 128], bf16, tag="tr")
                            nc.tensor.transpose(
                                pt[:], oh[:, k * 128:(k + 1) * 128], idb[:])
                            if k % 2 == 0:
                                nc.scalar.copy(
                                    ohT[:, k * 128:(k + 1) * 128], pt[:])
                            else:
                                nc.vector.tensor_copy(
                                    ohT[:, k * 128:(k + 1) * 128], pt[:])

                        # al_dst broadcast to edge slots
                        ps_e = psE.tile([128, KT * HEADS], f32, tag="attn")
                        for k in range(KT):
                            nc.tensor.matmul(
                                ps_e[:, k * HEADS:(k + 1) * HEADS],
                                lhsT=ohT[:, k * 128:(k + 1) * 128],
                                rhs=alts[t][:],
                                start=(k == 0), stop=(k == KT - 1))

                        # ex = exp(leaky_relu(al_src + al_dst))
                        e1 = wk.tile([128, KT * HEADS], f32, tag="e1")
                        e1v = e1[:].rearrange("p (k h) -> p k h", h=HEADS)
                        nc.vector.tensor_tensor(
                            e1v, g1v[:, :, D1:D1 + HEADS],
                            ps_e[:].rearrange("p (k h) -> p k h", h=HEADS),
                            op=ALU.add)
                        lr = wk.tile([128, KT * HEADS], f32, tag="lr")
                        nc.vector.tensor_scalar_mul(lr[:], e1[:], 0.2)
                        nc.vector.tensor_max(lr[:], lr[:], e1[:])
                        exw = wk.tile([128, KT * HEADS], bf16, tag="exw")
                        nc.scalar.activation(exw[:], lr[:], AF.Exp)
                        exv = exw[:].rearrange("p (k h) -> p k h", h=HEADS)

                        # weight gathered rows; ex into the al_src cols (denom)
                        g1f = g1v[:, :, 0:D1].rearrange(
                            "p k (h c) -> p k h c", c=HID)
                        exb = exv.unsqueeze(3).to_broadcast([128, KT, HEADS, HID])
                        nc.vector.tensor_mul(g1f, g1f, exb)
                        nc.vector.tensor_copy(g1v[:, :, D1:D1 + HEADS], exv)

                        # self-loop path: own rows via one contiguous DMA
                        own = wk.tile([128, G1W], bf16, tag="own")
                        nc.sync.dma_start(out=own[:],
                                          in_=loc1[t * 128:(t + 1) * 128, 0:G1W])
                        es = wk.tile([128, HEADS], f32, tag="es")
                        nc.vector.tensor_tensor(es[:], own[:, D1:D1 + HEADS],
                                                alts[t][:], op=ALU.add)
                        ls = wk.tile([128, HEADS], f32, tag="ls")
                        nc.vector.tensor_scalar_mul(ls[:], es[:], 0.2)
                        nc.vector.tensor_max(ls[:], ls[:], es[:])
                        exse = wk.tile([128, HEADS], bf16, tag="exse")
                        nc.scalar.activation(exse[:], ls[:], AF.Exp)
                        tmp = wk.tile([128, G1W], bf16, tag="tmps")
                        tmpv = tmp[:, 0:D1].rearrange("p (h c) -> p h c", c=HID)
                        exsb = exse[:].unsqueeze(2).to_broadcast([128, HEADS, HID])
                        nc.vector.tensor_mul(
                            tmpv,
                            own[:, 0:D1].rearrange("p (h c) -> p h c", c=HID),
                            exsb)
                        nc.vector.tensor_copy(tmp[:, D1:D1 + HEADS], exse[:])

                        # scatter-add rows + denominators (self first)
                        ps_c = psC.tile([128, G1W], f32, tag="scat")
                        nc.tensor.matmul(ps_c[:], lhsT=idb[:], rhs=tmp[:],
                                         start=True, stop=False)
                        for k in range(KT):
                            nc.tensor.matmul(
                                ps_c[:],
                                lhsT=oh[:, k * 128:(k + 1) * 128],
                                rhs=g1v[:, k, 0:G1W],
                                start=False, stop=(k == KT - 1))

                        den = wk.tile([128, HEADS], f32, tag="den")
                        nc.vector.tensor_scalar_add(den[:], ps_c[:, D1:D1 + HEADS],
                                                    1e-16)
                        rec = wk.tile([128, HEADS], f32, tag="rec")
                        nc.vector.reciprocal(rec[:], den[:])

                        o1 = wk.tile([128, D1], f32, tag="o1")
                        o1v = o1[:].rearrange("p (h c) -> p h c", c=HID)
                        recb = rec[:].unsqueeze(2).to_broadcast([128, HEADS, HID])
                        psf = ps_c[:, 0:D1].rearrange("p (h c) -> p h c", c=HID)
                        nc.vector.tensor_mul(o1v, psf, recb)
                        nc.vector.tensor_add(o1[:], o1[:], b1s[:])
                        # elu(x)+1 = relu(x) + exp(x - relu(x)); -1 folded in W2
                        rl_ = wk.tile([128, D1], f32, tag="rl_")
                        nc.scalar.activation(rl_[:], o1[:], AF.Relu)
                        tn = wk.tile([128, D1], f32, tag="tn")
                        nc.vector.tensor_tensor(tn[:], o1[:], rl_[:],
                                                op=ALU.subtract)
                        nc.scalar.activation(tn[:], tn[:], AF.Exp)
                        o1b = wk.tile([128, D1], bf16, tag="o1b")
                        nc.vector.tensor_add(o1b[:], rl_[:], tn[:])

                        # fused layer-2 local matmul: [z2 | al_src2 | al_dst2]
                        tts = []
                        for kk in range(DK):
                            pt2 = psT.tile([128, 128], bf16, tag="tr")
                            nc.tensor.transpose(
                                pt2[:], o1b[:, kk * 128:(kk + 1) * 128], idb[:])
                            tt = wk.tile([128, 128], bf16, tag=f"tt{kk}")
                            nc.scalar.copy(tt[:], pt2[:])
                            tts.append(tt)
                        ps_d = psD.tile([128, TW2], f32, tag="mmD")
                        for kk in range(DK):
                            nc.tensor.matmul(ps_d[:], lhsT=tts[kk][:],
                                             rhs=w2t[kk][:],
                                             start=(kk == 0), stop=False)
                        nc.tensor.matmul(ps_d[:], lhsT=one[:], rhs=w2cs[:],
                                         start=False, stop=True)
                        hd = wk.tile([128, T2W], bf16, tag="hd")
                        nc.scalar.copy(hd[:, 0:G2W], ps_d[:, 0:G2W])
                        nc.vector.memset(hd[:, G2W:T2W], 0.0)
                        alt2 = altp.tile([128, 1], bf16, tag=f"alt2_{t}")
                        nc.scalar.copy(alt2[:], ps_d[:, G2W:G2W + 1])
                        alt2s.append(alt2)
                        nc.sync.dma_start(out=loc2[t * 128:(t + 1) * 128, :],
                                          in_=hd[:])

            # ---- phase E: allgather layer-2 table ----
            with nc.named_scope("l2_allgather"):
                nc.gpsimd.collective_compute(
                    "AllGather", mybir.AluOpType.bypass, replica_groups=rg,
                    ins=[loc2[:]], outs=[tab2[:]],
                )
            tc.strict_bb_all_engine_barrier()

            # ---- phase F: layer-2 edge pass + fused log-softmax ----
            with nc.named_scope("l2_edges"):
                with (
                    tc.tile_pool(name="psT2", bufs=2, space="PSUM") as psT2,
                    tc.tile_pool(name="psE2", bufs=2, space="PSUM") as psE2,
                    tc.tile_pool(name="psC2", bufs=2, space="PSUM") as psC2,
                ):
                    for t in range(NT):
                        g2 = gp.tile([128, KT * T2W], bf16, tag="g2")
                        g2v = g2[:].rearrange("p (k c) -> p k c", c=T2W)
                        ib = t * KT * 8
                        nc.gpsimd.dma_gather(
                            g2v[:, 0:KL, :], tab2[0:LOWR, :],
                            ixs[:, ib:ib + KL * 8], KL * 128, KL * 128, T2W,
                            single_packet=False)
                        nc.gpsimd.dma_gather(
                            g2v[:, KL:KT, :], tab2[LOWR:NPALL, :],
                            ixs[:, ib + KL * 8:ib + KT * 8], KH * 128, KH * 128,
                            T2W, single_packet=False)

                        oh = wk.tile([128, KT * 128], bf16, tag="oh")
                        ohv = oh[:].rearrange("p (k j) -> p k j", j=128)
                        dlb = dlv[:, t, :].unsqueeze(2).to_broadcast([128, KT, 128])
                        iobb = iob[:].unsqueeze(1).to_broadcast([128, KT, 128])
                        nc.vector.tensor_tensor(ohv, dlb, iobb, op=ALU.is_equal)
                        ohT = wk.tile([128, KT * 128], bf16, tag="ohT")
                        for k in range(KT):
                            pt = psT2.tile([128, 128], bf16, tag="tr")
                            nc.tensor.transpose(
                                pt[:], oh[:, k * 128:(k + 1) * 128], idb[:])
                            if k % 2 == 0:
                                nc.scalar.copy(
                                    ohT[:, k * 128:(k + 1) * 128], pt[:])
                            else:
                                nc.vector.tensor_copy(
                                    ohT[:, k * 128:(k + 1) * 128], pt[:])

                        ps_e2 = psE2.tile([128, KT], f32, tag="attn2")
                        for k in range(KT):
                            nc.tensor.matmul(
                                ps_e2[:, k:k + 1],
                                lhsT=ohT[:, k * 128:(k + 1) * 128],
                                rhs=alt2s[t][:],
                                start=(k == 0), stop=(k == KT - 1))

                        e2 = wk.tile([128, KT], f32, tag="e2")
                        nc.vector.tensor_tensor(e2[:], g2v[:, :, NCLS], ps_e2[:],
                                                op=ALU.add)
                        lr2 = wk.tile([128, KT], f32, tag="lr2")
                        nc.vector.tensor_scalar_mul(lr2[:], e2[:], 0.2)
                        nc.vector.tensor_max(lr2[:], lr2[:], e2[:])
                        ex2 = wk.tile([128, KT], bf16, tag="ex2")
                        nc.scalar.activation(ex2[:], lr2[:], AF.Exp)

                        g2f = g2v[:, :, 0:NCLS]
                        ex2b = ex2[:].unsqueeze(2).to_broadcast([128, KT, NCLS])
                        nc.vector.tensor_mul(g2f, g2f, ex2b)
                        nc.vector.tensor_copy(g2v[:, :, NCLS], ex2[:])

                        own2 = wk.tile([128, G2W], bf16, tag="own2")
                        nc.sync.dma_start(out=own2[:],
                                          in_=loc2[t * 128:(t + 1) * 128, 0:G2W])
                        es2 = wk.tile([128, 1], f32, tag="es2")
                        nc.vector.tensor_tensor(es2[:], own2[:, NCLS:NCLS + 1],
                                                alt2s[t][:], op=ALU.add)
                        ls2 = wk.tile([128, 1], f32, tag="ls2")
                        nc.vector.tensor_scalar_mul(ls2[:], es2[:], 0.2)
                        nc.vector.tensor_max(ls2[:], ls2[:], es2[:])
                        exs2 = wk.tile([128, 1], bf16, tag="exs2")
                        nc.scalar.activation(exs2[:], ls2[:], AF.Exp)
                        tmp2 = wk.tile([128, G2W], bf16, tag="tmps2")
                        exs2b = exs2[:].to_broadcast([128, NCLS])
                        nc.vector.tensor_mul(tmp2[:, 0:NCLS], own2[:, 0:NCLS],
                                             exs2b)
                        nc.vector.tensor_copy(tmp2[:, NCLS:NCLS + 1], exs2[:])

                        ps_f = psC2.tile([128, G2W], f32, tag="scat2")
                        nc.tensor.matmul(ps_f[:], lhsT=idb[:], rhs=tmp2[:],
                                         start=True, stop=False)
                        for k in range(KT):
                            nc.tensor.matmul(
                                ps_f[:],
                                lhsT=oh[:, k * 128:(k + 1) * 128],
                                rhs=g2v[:, k, 0:G2W],
                                start=False, stop=(k == KT - 1))

                        den2 = wk.tile([128, 1], f32, tag="den2")
                        nc.vector.tensor_scalar_add(den2[:], ps_f[:, NCLS:NCLS + 1],
                                                    1e-16)
                        rec2 = wk.tile([128, 1], f32, tag="rec2")
                        nc.vector.reciprocal(rec2[:], den2[:])

                        o2 = wk.tile([128, NCLS], f32, tag="o2")
                        nc.vector.tensor_scalar(o2[:], ps_f[:, 0:NCLS],
                                                rec2[:, 0:1], None, op0=ALU.mult)
                        nc.vector.tensor_add(o2[:], o2[:], b2s[:])

                        # fused log-softmax
                        rmax = wk.tile([128, 1], f32, tag="rmax")
                        nc.vector.reduce_max(rmax[:], o2[:], axis=AX.X)
                        nrm = wk.tile([128, 1], f32, tag="nrm")
                        nc.vector.tensor_scalar_mul(nrm[:], rmax[:], -1.0)
                        exl = wk.tile([128, NCLS], f32, tag="exl")
                        ssum = wk.tile([128, 1], f32, tag="ssum")
                        nc.scalar.activation(exl[:], o2[:], AF.Exp,
                                             bias=nrm[:, 0:1],
                                             accum_out=ssum[:, 0:1])
                        lg = wk.tile([128, 1], f32, tag="lg")
                        nc.scalar.activation(lg[:], ssum[:], AF.Ln)
                        nb = wk.tile([128, 1], f32, tag="nb")
                        nc.vector.tensor_scalar(nb[:], rmax[:], lg[:, 0:1], -1.0,
                                                op0=ALU.add, op1=ALU.mult)
                        outf = wk.tile([128, NCLS], f32, tag="outf")
                        nc.scalar.activation(outf[:], o2[:], AF.Identity,
                                             bias=nb[:, 0:1])
                        nc.sync.dma_start(out=outp[t * 128:(t + 1) * 128, :],
                                          in_=outf[:])

    nc.compile()
    return nc


def _get_program(cfg):
    key = tuple(sorted(cfg.items()))
    if key not in _BUILD_CACHE:
        _BUILD_CACHE[key] = _build_program(**cfg)
    return _BUILD_CACHE[key]


def kernel(**inputs):
    C = 8
    cfg, in_maps, node_at, (N, NCLS) = _host_prep(
        inputs["x"], inputs["edge_index"], inputs["W1"], inputs["a_src1"],
        inputs["a_dst1"], inputs["b1"], inputs["W2"], inputs["a_src2"],
        inputs["a_dst2"], inputs["b2"], C,
    )
    nc = _get_program(cfg)

    from concourse.bass_utils import run_bass_kernel_spmd

    trace = bool(int(os.environ.get("GAT_PROFILE", "0")))
    if trace:
        trace = _register_trace_hook()
    res = run_bass_kernel_spmd(nc, in_maps, list(range(C)), trace=trace)
    if trace and res.exec_time_ns is not None:
        print(f"HW exec time: {res.exec_time_ns} ns", flush=True)

    out = np.empty((N, NCLS), np.float32)
    for c in range(C):
        r = res.results[c]["outp"]
        m = node_at[c] >= 0
        out[node_at[c][m]] = r[m]
    return out


# revision 8
# speedup vs baseline: 2.2020x; 1.1423x over previous
"""Distributed 2-layer GAT on 8 Trainium2 NeuronCores (Bass/Tile).

Strategy (graph/data parallel, per sharding hint):
  - Nodes sharded across 8 cores (6250 each, padded to 6272 = 49*128); nodes
    greedily packed into 49 tiles of 128 balancing per-tile in-edge counts.
  - Layer tables (bf16) computed locally ([h | al_src | al_dst] extended
    matmul) and AllGathered so every core holds the full node table in DRAM.
    Table rows padded to 256B multiples for the ucode gather (768B layer 1,
    256B layer 2).
  - Edge pass per destination tile: one ucode dma_gather per (tile, src-range
    half) pulls all remote source rows (int16 indices force a <32768 / >=32768
    range split; edges sorted by source row so gather columns are range-pure;
    trailing -1 indices + per-core exact counts loaded into a gpsimd register
    skip pad descriptors). Self-loops bypass the gather: the tile's own rows
    are fetched with one contiguous DMA and accumulated via an identity
    matmul. Attention: al_dst broadcast to edge slots via transposed one-hot
    matmuls (one-hot built on-chip, PE-transposed). Unnormalized softmax
    ex = exp(leaky_relu(al_src+al_dst)) scales the gathered rows; a one-hot
    selection matmul scatter-adds rows and denominators into PSUM.
  - Pad edge slots keep dst-slot 999 -> zero one-hot column -> no
    contribution (gather buffers are zero-primed so skipped slots hold
    finite stale data).
  - Layer-2 local matmul fused into the layer-1 edge loop (elu(x)+1 fed
    through W2 with a -colsum(W2e) correction row); final log-softmax fused
    into the layer-2 edge loop.
"""

import heapq
import os
import sys
import types

import ml_dtypes
import numpy as np

_BUILD_CACHE = {}
LOWR = 32768


def _register_trace_hook():
    """Best-effort registration of the axon NTFF profiling hook."""
    try:
        if "antenv.axon_hooks" in sys.modules:
            return True
        from trn_agent_boot.trn_boot import _ntff_profile_via_ctypes

        hook = _ntff_profile_via_ctypes("/opt/axon/libaxon_pjrt.so")
        m = types.ModuleType("antenv.axon_hooks")
        m.get_axon_ntff_profile_hook = lambda: hook
        m.set_axon_ntff_profile_hook = lambda h: None
        sys.modules["antenv.axon_hooks"] = m
        return True
    except Exception:
        return False


def _host_prep(x, edge_index, W1, a_src1, a_dst1, b1, W2, a_src2, a_dst2, b2, C):
    bf16 = ml_dtypes.bfloat16
    x = np.asarray(x, np.float32)
    ei = np.asarray(edge_index)
    W1 = np.asarray(W1, np.float32)
    a_src1 = np.asarray(a_src1, np.float32)
    a_dst1 = np.asarray(a_dst1, np.float32)
    b1 = np.asarray(b1, np.float32)
    W2 = np.asarray(W2, np.float32)
    a_src2 = np.asarray(a_src2, np.float32)
    a_dst2 = np.asarray(a_dst2, np.float32)
    b2 = np.asarray(b2, np.float32)

    N, F = x.shape
    HEADS, HID = a_src1.shape
    D1 = HEADS * HID
    NCLS = W2.shape[1]
    assert N % C == 0
    NPC = N // C
    NT = -(-NPC // 128)
    PADN = NT * 128
    NPALL = C * PADN
    E = ei.shape[1]

    # self-loops are handled separately on-chip; only real edges here
    src = ei[0].astype(np.int64)
    dst = ei[1].astype(np.int64)

    # --- greedy degree-balanced node->tile assignment per core ---
    deg = np.bincount(dst, minlength=N).astype(np.int64)
    pos = np.empty(N, np.int64)
    for c in range(C):
        lo = c * NPC
        d = deg[lo:lo + NPC]
        order = np.argsort(-d, kind="stable")
        cnts_ = np.zeros(NT, np.int64)
        heap = [(0, t) for t in range(NT)]
        heapq.heapify(heap)
        ploc = np.empty(NPC, np.int64)
        for i in order:
            while True:
                load, t = heapq.heappop(heap)
                if cnts_[t] < 128:
                    break
            ploc[i] = t * 128 + cnts_[t]
            cnts_[t] += 1
            if cnts_[t] < 128:
                heapq.heappush(heap, (load + d[i], t))
        pos[lo:lo + NPC] = ploc

    ncidx = np.arange(N) // NPC
    node_at = np.full((C, PADN), -1, np.int64)
    node_at[ncidx, pos] = np.arange(N)
    grow = ncidx * PADN + pos  # global table row per node

    # --- edge grouping by (dst core, dst tile, src-range half), src-sorted ---
    ec = dst // NPC
    et = pos[dst] // 128
    srow = grow[src]
    half = (srow >= LOWR).astype(np.int64)
    gkey = (ec * NT + et) * 2 + half
    order_e = np.lexsort((srow, gkey))
    ks = gkey[order_e]
    cnt = np.bincount(ks, minlength=C * NT * 2).reshape(C, NT, 2)
    KL = int(-(-cnt[:, :, 0].max() // 128))
    KH = int(-(-cnt[:, :, 1].max() // 128))
    KT = KL + KH

    gstart = np.searchsorted(ks, np.arange(C * NT * 2))
    jj = np.arange(E) - gstart[ks]
    half_s = ks % 2
    col = jj // 128 + np.where(half_s, KL, 0)
    pp = jj % 128
    cc = ks // (2 * NT)
    tt = (ks // 2) % NT

    s_dst = dst[order_e]
    idxf = np.zeros((C, NT, KT, 128), np.int16)
    idxf[cc, tt, col, pp] = (srow[order_e] - np.where(half_s, LOWR, 0)).astype(np.int16)
    dlarr = np.full((C, NT, KT, 128), 999.0, np.float32)
    dlarr[cc, tt, col, pp] = (pos[s_dst] % 128).astype(np.float32)

    # wrapped int16 index layout: call index i -> [i % 16, i // 16]
    def wrap(a, K):  # a: [C, NT, K*128] -> [C, NT, 16, K*8]
        Cn, NTn, NI = a.shape
        w = np.zeros((Cn, NTn, 16, NI // 16), np.int16)
        i = np.arange(NI)
        w[:, :, i % 16, i // 16] = a
        return w

    wl = wrap(idxf[:, :, 0:KL, :].reshape(C, NT, KL * 128), KL)
    wh = wrap(idxf[:, :, KL:KT, :].reshape(C, NT, KH * 128), KH)
    wfull = np.concatenate([wl, wh], axis=3)  # [C, NT, 16, KT*8]
    wfull = wfull.transpose(0, 2, 1, 3).reshape(C, 16, NT * KT * 8)
    ixs = np.tile(wfull, (1, 8, 1))  # [C, 128, NT*KT*8]

    dls = np.ascontiguousarray(
        dlarr.transpose(0, 3, 1, 2).reshape(C, 128, NT * KT)
    ).astype(bf16)
    cnts = cnt.reshape(C, 1, NT * 2).astype(np.int32)

    # --- per-core per-tile transposed x blocks (pad rows zero) ---
    xs = np.zeros((C, PADN, F), np.float32)
    xs[ncidx, pos] = x
    FK = F // 128
    xsTt = np.ascontiguousarray(
        xs.reshape(C, NT, 128, FK, 128).transpose(0, 1, 3, 4, 2)
    ).astype(bf16)  # [C, NT, FK, 128(feat), 128(node)]

    # --- extended weights ---
    Wa_s1 = np.einsum("fhc,hc->fh", W1.reshape(F, HEADS, HID), a_src1)
    Wa_d1 = np.einsum("fhc,hc->fh", W1.reshape(F, HEADS, HID), a_dst1)
    W1e = np.concatenate([W1, Wa_s1, Wa_d1], axis=1).astype(bf16)
    Wa_s2 = W2 @ a_src2[0]
    Wa_d2 = W2 @ a_dst2[0]
    W2e_f = np.concatenate([W2, Wa_s2[:, None], Wa_d2[:, None]], axis=1)
    W2e = W2e_f.astype(bf16)
    w2corr = (-W2e_f.sum(axis=0, keepdims=True)).astype(bf16)

    b1r = np.ascontiguousarray(np.broadcast_to(b1[None, :], (128, D1)), np.float32)
    b2r = np.ascontiguousarray(np.broadcast_to(b2[None, :], (128, NCLS)), np.float32)
    iotab = np.ascontiguousarray(
        np.broadcast_to(np.arange(128, dtype=np.float32)[None, :], (128, 128))
    ).astype(bf16)
    identb = np.eye(128, dtype=np.float32).astype(bf16)
    onesb = np.ones((1, 128), np.float32).astype(bf16)

    cfg = dict(F=F, HEADS=HEADS, HID=HID, D1=D1, NCLS=NCLS, NT=NT, PADN=PADN,
               NPALL=NPALL, KL=KL, KH=KH, C=C)
    in_maps = []
    for c in range(C):
        in_maps.append({
            "xsTt": xsTt[c],
            "w1e": W1e,
            "w2e": W2e,
            "w2c": w2corr,
            "b1r": b1r,
            "b2r": b2r,
            "iotab": iotab,
            "identb": identb,
            "onesb": onesb,
            "ixs": ixs[c],
            "dls": dls[c],
            "cnts": cnts[c],
        })
    return cfg, in_maps, node_at, (N, NCLS)


def _build_program(F, HEADS, HID, D1, NCLS, NT, PADN, NPALL, KL, KH, C):
    import concourse.bacc as bacc
    import concourse.bass as bass
    import concourse.mybir as mybir
    import concourse.tile as tile

    f32 = mybir.dt.float32
    bf16 = mybir.dt.bfloat16
    i16 = mybir.dt.int16
    i32 = mybir.dt.int32
    AF = mybir.ActivationFunctionType
    ALU = mybir.AluOpType
    AX = mybir.AxisListType

    KT = KL + KH
    TW1 = D1 + 2 * HEADS          # 264: [h | al_src | al_dst]
    G1W = D1 + HEADS              # 260: gathered layer-1 row payload
    T1W = 384                     # layer-1 table row (768B, 256B multiple)
    TW2 = NCLS + 2                # 66: [z2 | al_src2 | al_dst2]
    G2W = NCLS + 1                # 65: gathered layer-2 row payload
    T2W = 128                     # layer-2 table row (256B)
    FK = F // 128
    DK = D1 // 128

    nc = bacc.Bacc("TRN2", target_bir_lowering=False, debug=False, num_devices=C)

    xsTt = nc.dram_tensor("xsTt", [NT, FK, 128, 128], bf16, kind="ExternalInput")
    w1e = nc.dram_tensor("w1e", [F, TW1], bf16, kind="ExternalInput")
    w2e = nc.dram_tensor("w2e", [D1, TW2], bf16, kind="ExternalInput")
    w2c = nc.dram_tensor("w2c", [1, TW2], bf16, kind="ExternalInput")
    b1r = nc.dram_tensor("b1r", [128, D1], f32, kind="ExternalInput")
    b2r = nc.dram_tensor("b2r", [128, NCLS], f32, kind="ExternalInput")
    iotab = nc.dram_tensor("iotab", [128, 128], bf16, kind="ExternalInput")
    identb = nc.dram_tensor("identb", [128, 128], bf16, kind="ExternalInput")
    onesb = nc.dram_tensor("onesb", [1, 128], bf16, kind="ExternalInput")
    ixsd = nc.dram_tensor("ixs", [128, NT * KT * 8], i16, kind="ExternalInput")
    dlsd = nc.dram_tensor("dls", [128, NT * KT], bf16, kind="ExternalInput")
    cntd = nc.dram_tensor("cnts", [1, NT * 2], i32, kind="ExternalInput")
    outp = nc.dram_tensor("outp", [PADN, NCLS], f32, kind="ExternalOutput")

    loc1 = nc.dram_tensor("loc1", [PADN, T1W], bf16)
    tab1 = nc.dram_tensor("tab1", [NPALL, T1W], bf16)
    loc2 = nc.dram_tensor("loc2", [PADN, T2W], bf16)
    tab2 = nc.dram_tensor("tab2", [NPALL, T2W], bf16)

    rg = [list(range(C))]

    with tile.TileContext(nc) as tc:
        with (
            tc.tile_pool(name="const", bufs=1) as const,
            tc.tile_pool(name="altp", bufs=1) as altp,
            tc.tile_pool(name="wk", bufs=2) as wk,
            tc.tile_pool(name="gp", bufs=4) as gp,
        ):
            # ---- constants ----
            w1t = []
            for kk in range(FK):
                t_ = const.tile([128, TW1], bf16, tag=f"w1_{kk}")
                nc.sync.dma_start(out=t_[:], in_=w1e[kk * 128:(kk + 1) * 128, :])
                w1t.append(t_)
            w2t = []
            for kk in range(DK):
                t_ = const.tile([128, TW2], bf16, tag=f"w2_{kk}")
                nc.sync.dma_start(out=t_[:], in_=w2e[kk * 128:(kk + 1) * 128, :])
                w2t.append(t_)
            w2cs = const.tile([1, TW2], bf16, tag="w2c")
            nc.sync.dma_start(out=w2cs[:], in_=w2c[:, :])
            b1s = const.tile([128, D1], f32, tag="b1")
            nc.sync.dma_start(out=b1s[:], in_=b1r[:, :])
            b2s = const.tile([128, NCLS], f32, tag="b2")
            nc.sync.dma_start(out=b2s[:], in_=b2r[:, :])
            iob = const.tile([128, 128], bf16, tag="iota")
            nc.sync.dma_start(out=iob[:], in_=iotab[:, :])
            idb = const.tile([128, 128], bf16, tag="ident")
            nc.sync.dma_start(out=idb[:], in_=identb[:, :])
            one = const.tile([1, 128], bf16, tag="ones")
            nc.sync.dma_start(out=one[:], in_=onesb[:, :])
            ixs = const.tile([128, NT * KT * 8], i16, tag="ixs")
            nc.sync.dma_start(out=ixs[:], in_=ixsd[:, :])
            dls = const.tile([128, NT * KT], bf16, tag="dls")
            nc.sync.dma_start(out=dls[:], in_=dlsd[:, :])
            dlv = dls[:].rearrange("p (t k) -> p t k", k=KT)
            cn = const.tile([1, NT * 2], i32, tag="cnts")
            nc.sync.dma_start(out=cn[:], in_=cntd[:, :])
            alts = []
            alt2s = []

            # ---- phase A: local [h | al_src | al_dst] = x @ W1e ----
            with nc.named_scope("l1_local_mm"):
                with tc.tile_pool(name="psA", bufs=2, space="PSUM") as psA:
                    for t in range(NT):
                        ps_a = psA.tile([128, TW1], f32, tag="mm")
                        for kk in range(FK):
                            xt = wk.tile([128, 128], bf16, tag=f"xt{kk}")
                            nc.sync.dma_start(out=xt[:], in_=xsTt[t, kk])
                            nc.tensor.matmul(ps_a[:], lhsT=xt[:], rhs=w1t[kk][:],
                                             start=(kk == 0), stop=(kk == FK - 1))
                        ha = wk.tile([128, T1W], bf16, tag="ha")
                        nc.scalar.copy(ha[:, 0:G1W], ps_a[:, 0:G1W])
                        nc.vector.memset(ha[:, G1W:T1W], 0.0)
                        alt = altp.tile([128, HEADS], bf16, tag=f"alt{t}")
                        nc.scalar.copy(alt[:], ps_a[:, D1 + HEADS:D1 + 2 * HEADS])
                        alts.append(alt)
                        nc.sync.dma_start(out=loc1[t * 128:(t + 1) * 128, :],
                                          in_=ha[:])

            # ---- phase B: allgather layer-1 table ----
            with nc.named_scope("l1_allgather"):
                nc.gpsimd.collective_compute(
                    "AllGather", mybir.AluOpType.bypass, replica_groups=rg,
                    ins=[loc1[:]], outs=[tab1[:]],
                )
            tc.strict_bb_all_engine_barrier()

            # ---- phase C: layer-1 edge pass + fused layer-2 local matmul ----
            with nc.named_scope("l1_edges"):
                with (
                    tc.tile_pool(name="psT", bufs=2, space="PSUM") as psT,
                    tc.tile_pool(name="psE", bufs=2, space="PSUM") as psE,
                    tc.tile_pool(name="psC", bufs=2, space="PSUM") as psC,
                    tc.tile_pool(name="psD", bufs=2, space="PSUM") as psD,
                ):
                    for t in range(NT):
                        g1 = gp.tile([128, KT * T1W], bf16, tag="g1")
                        g1v = g1[:].rearrange("p (k c) -> p k c", c=T1W)
                        ib = t * KT * 8
                        nc.gpsimd.dma_gather(
                            g1v[:, 0:KL, :], tab1[0:LOWR, :],
                            ixs[:, ib:ib + KL * 8], KL * 128, KL * 128, T1W,
                            single_packet=False)
                        nc.gpsimd.dma_gather(
                            g1v[:, KL:KT, :], tab1[LOWR:NPALL, :],
                            ixs[:, ib + KL * 8:ib + KT * 8], KH * 128, KH * 128,
                            T1W, single_packet=False)

                        # one-hot (edge slot p -> dst row j) and its transpose
                        oh = wk.tile([128, KT * 128], bf16, tag="oh")
                        ohv = oh[:].rearrange("p (k j) -> p k j", j=128)
                        dlb = dlv[:, t, :].unsqueeze(2).to_broadcast([128, KT, 128])
                        iobb = iob[:].unsqueeze(1).to_broadcast([128, KT, 128])
                        nc.vector.tensor_tensor(ohv, dlb, iobb, op=ALU.is_equal)
                        ohT = wk.tile([128, KT * 128], bf16, tag="ohT")
                        for k in range(KT):
                            pt = psT.tile([128, 128], bf16, tag="tr")
                            nc.tensor.transpose(
                                pt[:], oh[:, k * 128:(k + 1) * 128], idb[:])
                            nc.scalar.copy(ohT[:, k * 128:(k + 1) * 128],
                                           pt[:])

                        # al_dst broadcast to edge slots
                        ps_e = psE.tile([128, KT * HEADS], f32, tag="attn")
                        for k in range(KT):
                            nc.tensor.matmul(
                                ps_e[:, k * HEADS:(k + 1) * HEADS],
                                lhsT=ohT[:, k * 128:(k + 1) * 128],
                                rhs=alts[t][:],
                                start=(k == 0), stop=(k == KT - 1))

                        # ex = exp(leaky_relu(al_src + al_dst))
                        e1 = wk.tile([128, KT * HEADS], f32, tag="e1")
                        e1v = e1[:].rearrange("p (k h) -> p k h", h=HEADS)
                        nc.vector.tensor_tensor(
                            e1v, g1v[:, :, D1:D1 + HEADS],
                            ps_e[:].rearrange("p (k h) -> p k h", h=HEADS),
                            op=ALU.add)
                        lr = wk.tile([128, KT * HEADS], f32, tag="lr")
                        nc.vector.tensor_scalar_mul(lr[:], e1[:], 0.2)
                        nc.vector.tensor_max(lr[:], lr[:], e1[:])
                        exw = wk.tile([128, KT * HEADS], bf16, tag="exw")
                        nc.scalar.activation(exw[:], lr[:], AF.Exp)
                        exv = exw[:].rearrange("p (k h) -> p k h", h=HEADS)

                        # weight gathered rows; ex into the al_src cols (denom)
                        g1f = g1v[:, :, 0:D1].rearrange(
                            "p k (h c) -> p k h c", c=HID)
                        exb = exv.unsqueeze(3).to_broadcast([128, KT, HEADS, HID])
                        nc.vector.tensor_mul(g1f, g1f, exb)
                        nc.scalar.copy(g1v[:, :, D1:D1 + HEADS], exv)

                        # self-loop path: own rows via one contiguous DMA
                        own = wk.tile([128, G1W], bf16, tag="own")
                        nc.sync.dma_start(out=own[:],
                                          in_=loc1[t * 128:(t + 1) * 128, 0:G1W])
                        es = wk.tile([128, HEADS], f32, tag="es")
                        nc.vector.tensor_tensor(es[:], own[:, D1:D1 + HEADS],
                                                alts[t][:], op=ALU.add)
                        ls = wk.tile([128, HEADS], f32, tag="ls")
                        nc.vector.tensor_scalar_mul(ls[:], es[:], 0.2)
                        nc.vector.tensor_max(ls[:], ls[:], es[:])
                        exse = wk.tile([128, HEADS], bf16, tag="exse")
                        nc.scalar.activation(exse[:], ls[:], AF.Exp)
                        tmp = wk.tile([128, G1W], bf16, tag="tmps")
                        tmpv = tmp[:, 0:D1].rearrange("p (h c) -> p h c", c=HID)
                        exsb = exse[:].unsqueeze(2).to_broadcast([128, HEADS, HID])
                        nc.vector.tensor_mul(
                            tmpv,
                            own[:, 0:D1].rearrange("p (h c) -> p h c", c=HID),
                            exsb)
                        nc.scalar.copy(tmp[:, D1:D1 + HEADS], exse[:])

                        # scatter-add rows + denominators (self first)
                        ps_c = psC.tile([128, G1W], f32, tag="scat")
                        nc.tensor.matmul(ps_c[:], lhsT=idb[:], rhs=tmp[:],
                                         start=True, stop=False)
                        for k in range(KT):
                            nc.tensor.matmul(
                                ps_c[:],
                                lhsT=oh[:, k * 128:(k + 1) * 128],
                                rhs=g1v[:, k, 0:G1W],
                                start=False, stop=(k == KT - 1))

                        den = wk.tile([128, HEADS], f32, tag="den")
                        nc.vector.tensor_scalar_add(den[:], ps_c[:, D1:D1 + HEADS],
                                                    1e-16)
                        rec = wk.tile([128, HEADS], f32, tag="rec")
                        nc.vector.reciprocal(rec[:], den[:])

                        o1 = wk.tile([128, D1], f32, tag="o1")
                        o1v = o1[:].rearrange("p (h c) -> p h c", c=HID)
                        recb = rec[:].unsqueeze(2).to_broadcast([128, HEADS, HID])
                        psf = ps_c[:, 0:D1].rearrange("p (h c) -> p h c", c=HID)
                        nc.vector.tensor_mul(o1v, psf, recb)
                        nc.vector.tensor_add(o1[:], o1[:], b1s[:])
                        # elu(x)+1 = relu(x) + exp(x - relu(x)); -1 folded in W2
                        rl_ = wk.tile([128, D1], f32, tag="rl_")
                        nc.scalar.activation(rl_[:], o1[:], AF.Relu)
                        tn = wk.tile([128, D1], f32, tag="tn")
                        nc.vector.tensor_tensor(tn[:], o1[:], rl_[:],
                                                op=ALU.subtract)
                        nc.scalar.activation(tn[:], tn[:], AF.Exp)
                        o1b = wk.tile([128, D1], bf16, tag="o1b")
                        nc.vector.tensor_add(o1b[:], rl_[:], tn[:])

                        # fused layer-2 local matmul: [z2 | al_src2 | al_dst2]
                        tts = []
                        for kk in range(DK):
                            pt2 = psT.tile([128, 128], bf16, tag="tr")
                            nc.tensor.transpose(
                                pt2[:], o1b[:, kk * 128:(kk + 1) * 128], idb[:])
                            tt = wk.tile([128, 128], bf16, tag=f"tt{kk}")
                            nc.scalar.copy(tt[:], pt2[:])
                            tts.append(tt)
                        ps_d = psD.tile([128, TW2], f32, tag="mmD")
                        for kk in range(DK):
                            nc.tensor.matmul(ps_d[:], lhsT=tts[kk][:],
                                             rhs=w2t[kk][:],
                                             start=(kk == 0), stop=False)
                        nc.tensor.matmul(ps_d[:], lhsT=one[:], rhs=w2cs[:],
                                         start=False, stop=True)
                        hd = wk.tile([128, T2W], bf16, tag="hd")
                        nc.scalar.copy(hd[:, 0:G2W], ps_d[:, 0:G2W])
                        nc.vector.memset(hd[:, G2W:T2W], 0.0)
                        alt2 = altp.tile([128, 1], bf16, tag=f"alt2_{t}")
                        nc.scalar.copy(alt2[:], ps_d[:, G2W:G2W + 1])
                        alt2s.append(alt2)
                        nc.sync.dma_start(out=loc2[t * 128:(t + 1) * 128, :],
                                          in_=hd[:])

            # ---- phase E: allgather layer-2 table ----
            with nc.named_scope("l2_allgather"):
                nc.gpsimd.collective_compute(
                    "AllGather", mybir.AluOpType.bypass, replica_groups=rg,
                    ins=[loc2[:]], outs=[tab2[:]],
                )
            tc.strict_bb_all_engine_barrier()

            # ---- phase F: layer-2 edge pass + fused log-softmax ----
            with nc.named_scope("l2_edges"):
                with (
                    tc.tile_pool(name="psT2", bufs=2, space="PSUM") as psT2,
                    tc.tile_pool(name="psE2", bufs=2, space="PSUM") as psE2,
                    tc.tile_pool(name="psC2", bufs=2, space="PSUM") as psC2,
                ):
                    for t in range(NT):
                        g2 = gp.tile([128, KT * T2W], bf16, tag="g2")
                        g2v = g2[:].rearrange("p (k c) -> p k c", c=T2W)
                        ib = t * KT * 8
                        nc.gpsimd.dma_gather(
                            g2v[:, 0:KL, :], tab2[0:LOWR, :],
                            ixs[:, ib:ib + KL * 8], KL * 128, KL * 128, T2W,
                            single_packet=False)
                        nc.gpsimd.dma_gather(
                            g2v[:, KL:KT, :], tab2[LOWR:NPALL, :],
                            ixs[:, ib + KL * 8:ib + KT * 8], KH * 128, KH * 128,
                            T2W, single_packet=False)

                        oh = wk.tile([128, KT * 128], bf16, tag="oh")
                        ohv = oh[:].rearrange("p (k j) -> p k j", j=128)
                        dlb = dlv[:, t, :].unsqueeze(2).to_broadcast([128, KT, 128])
                        iobb = iob[:].unsqueeze(1).to_broadcast([128, KT, 128])
                        nc.vector.tensor_tensor(ohv, dlb, iobb, op=ALU.is_equal)
                        ohT = wk.tile([128, KT * 128], bf16, tag="ohT")
                        for k in range(KT):
                            pt = psT2.tile([128, 128], bf16, tag="tr")
                            nc.tensor.transpose(
                                pt[:], oh[:, k * 128:(k + 1) * 128], idb[:])
                            nc.scalar.copy(ohT[:, k * 128:(k + 1) * 128],
                                           pt[:])

                        ps_e2 = psE2.tile([128, KT], f32, tag="attn2")
                        for k in range(KT):
                            nc.tensor.matmul(
                                ps_e2[:, k:k + 1],
                                lhsT=ohT[:, k * 128:(k + 1) * 128],
                                rhs=alt2s[t][:],
                                start=(k == 0), stop=(k == KT - 1))

                        e2 = wk.tile([128, KT], f32, tag="e2")
                        nc.vector.tensor_tensor(e2[:], g2v[:, :, NCLS], ps_e2[:],
                                                op=ALU.add)
                        lr2 = wk.tile([128, KT], f32, tag="lr2")
                        nc.vector.tensor_scalar_mul(lr2[:], e2[:], 0.2)
                        nc.vector.tensor_max(lr2[:], lr2[:], e2[:])
                        ex2 = wk.tile([128, KT], bf16, tag="ex2")
                        nc.scalar.activation(ex2[:], lr2[:], AF.Exp)

                        g2f = g2v[:, :, 0:NCLS]
                        ex2b = ex2[:].unsqueeze(2).to_broadcast([128, KT, NCLS])
                        nc.vector.tensor_mul(g2f, g2f, ex2b)
                        nc.scalar.copy(g2v[:, :, NCLS], ex2[:])

                        own2 = wk.tile([128, G2W], bf16, tag="own2")
                        nc.sync.dma_start(out=own2[:],
                                          in_=loc2[t * 128:(t + 1) * 128, 0:G2W])
                        es2 = wk.tile([128, 1], f32, tag="es2")
                        nc.vector.tensor_tensor(es2[:], own2[:, NCLS:NCLS + 1],
                                                alt2s[t][:], op=ALU.add)
                        ls2 = wk.tile([128, 1], f32, tag="ls2")
                        nc.vector.tensor_scalar_mul(ls2[:], es2[:], 0.2)
                        nc.vector.tensor_max(ls2[:], ls2[:], es2[:])
                        exs2 = wk.tile([128, 1], bf16, tag="exs2")
                        nc.scalar.activation(exs2[:], ls2[:], AF.Exp)
                        tmp2 = wk.tile([128, G2W], bf16, tag="tmps2")
                        exs2b = exs2[:].to_broadcast([128, NCLS])
                        nc.vector.tensor_mul(tmp2[:, 0:NCLS], own2[:, 0:NCLS],
                                             exs2b)
                        nc.scalar.copy(tmp2[:, NCLS:NCLS + 1], exs2[:])

                        ps_f = psC2.tile([128, G2W], f32, tag="scat2")
                        nc.tensor.matmul(ps_f[:], lhsT=idb[:], rhs=tmp2[:],
                                         start=True, stop=False)
                        for k in range(KT):
                            nc.tensor.matmul(
                                ps_f[:],
                                lhsT=oh[:, k * 128:(k + 1) * 128],
                                rhs=g2v[:, k, 0:G2W],
                                start=False, stop=(k == KT - 1))

                        den2 = wk.tile([128, 1], f32, tag="den2")
                        nc.vector.tensor_scalar_add(den2[:], ps_f[:, NCLS:NCLS + 1],
                                                    1e-16)
                        rec2 = wk.tile([128, 1], f32, tag="rec2")
                        nc.vector.reciprocal(rec2[:], den2[:])

                        o2 = wk.tile([128, NCLS], f32, tag="o2")
                        nc.vector.tensor_scalar(o2[:], ps_f[:, 0:NCLS],
                                                rec2[:, 0:1], None, op0=ALU.mult)
                        nc.vector.tensor_add(o2[:], o2[:], b2s[:])

                        # fused log-softmax
                        rmax = wk.tile([128, 1], f32, tag="rmax")
                        nc.vector.reduce_max(rmax[:], o2[:], axis=AX.X)
                        nrm = wk.tile([128, 1], f32, tag="nrm")
                        nc.vector.tensor_scalar_mul(nrm[:], rmax[:], -1.0)
                        exl = wk.tile([128, NCLS], f32, tag="exl")
                        ssum = wk.tile([128, 1], f32, tag="ssum")
                        nc.scalar.activation(exl[:], o2[:], AF.Exp,
                                             bias=nrm[:, 0:1],
                                             accum_out=ssum[:, 0:1])
                        lg = wk.tile([128, 1], f32, tag="lg")
                        nc.scalar.activation(lg[:], ssum[:], AF.Ln)
                        nb = wk.tile([128, 1], f32, tag="nb")
                        nc.vector.tensor_scalar(nb[:], rmax[:], lg[:, 0:1], -1.0,
                                                op0=ALU.add, op1=ALU.mult)
                        outf = wk.tile([128, NCLS], f32, tag="outf")
                        nc.scalar.activation(outf[:], o2[:], AF.Identity,
                                             bias=nb[:, 0:1])
                        nc.sync.dma_start(out=outp[t * 128:(t + 1) * 128, :],
                                          in_=outf[:])

    nc.compile()
    return nc


def _get_program(cfg):
    key = tuple(sorted(cfg.items()))
    if key not in _BUILD_CACHE:
        _BUILD_CACHE[key] = _build_program(**cfg)
    return _BUILD_CACHE[key]


def kernel(**inputs):
    C = 8
    cfg, in_maps, node_at, (N, NCLS) = _host_prep(
        inputs["x"], inputs["edge_index"], inputs["W1"], inputs["a_src1"],
        inputs["a_dst1"], inputs["b1"], inputs["W2"], inputs["a_src2"],
        inputs["a_dst2"], inputs["b2"], C,
    )
    nc = _get_program(cfg)

    from concourse.bass_utils import run_bass_kernel_spmd

    trace = bool(int(os.environ.get("GAT_PROFILE", "0")))
    if trace:
        trace = _register_trace_hook()
    res = run_bass_kernel_spmd(nc, in_maps, list(range(C)), trace=trace)
    if trace and res.exec_time_ns is not None:
        print(f"HW exec time: {res.exec_time_ns} ns", flush=True)

    out = np.empty((N, NCLS), np.float32)
    for c in range(C):
        r = res.results[c]["outp"]
        m = node_at[c] >= 0
        out[node_at[c][m]] = r[m]
    return out
